# revision 1
# baseline (speedup 1.0000x reference)
"""BEV voxel-pooling kernel for 8 TRN2 NeuronCores (data-parallel over batch).

Design (constraints established by on-HW probes):
  * dma_scatter_add loses duplicate indices WITHIN one instruction (every
    descriptor RMWs the pre-instruction value, last writer wins), but
    accumulates exactly ACROSS serialized instructions. Descriptor ring caps
    one instruction at ~1024 descriptors.
  => tokens are scattered in 1024-token windows; the host guarantees each
     window's live indices are unique (duplicate occurrences are diverted to a
     small overflow stream, re-windowed with the same property). Zero-payload
     tokens (out-of-grid, padding, diverted) are pointed at a slot that no
     live token of the same window uses, so their old+0 writes are harmless.

  Per core (1 batch): grid[32768 pair-slots, 128] f32; pair-slot =
  gx*128 + gy//2; a token's 64 features are placed in the low/high half of a
  512B payload by gy parity. Device: loads x (64MB), builds masked payloads
  on the DVE, runs the serialized scatter-add chain. Host: voxelization
  (exact f32 mirror of the reference math), window-dedup, final layout
  transpose of the device-produced voxel-major grid.
"""
import numpy as np

import concourse.bacc as bacc
import concourse.tile as tile
from concourse import mybir
from concourse.bass_utils import run_bass_kernel_spmd

F32 = mybir.dt.float32
I16 = mybir.dt.int16

B, N, D, H, W, C = 8, 6, 59, 16, 44, 64
NX, NY = 256, 256
NP = N * D * H * W            # 249216 points per batch/core
WIN = 1024                    # tokens per scatter instruction
NWIN = (NP + WIN - 1) // WIN  # 244
TOKPAD = NWIN * WIN           # 249856
COLS = TOKPAD // 128          # 1952
OF_BIG = 48                   # overflow: 48 full 1024-token windows (dense)
OF_SMALL = 256                # + 256 small 128-token windows (hot-slot tails)
OF_TOK = OF_BIG * WIN + OF_SMALL * 128   # 81920
OF_COLS = OF_TOK // 128       # 640
NSLOT = NX * (NY // 2)        # 32768 pair slots

_nc_cache = None


def _build():
    nc = bacc.Bacc("TRN2", target_bir_lowering=False, debug=False)
    xw = nc.dram_tensor("xw", [128, COLS, 64], F32, kind="ExternalInput")
    mpw = nc.dram_tensor("mpw", [128, COLS, 2], F32, kind="ExternalInput")
    idxw = nc.dram_tensor("idxw", [128, TOKPAD // 16], I16, kind="ExternalInput")
    ofx = nc.dram_tensor("ofx", [128, OF_COLS, 64], F32, kind="ExternalInput")
    ofmp = nc.dram_tensor("ofmp", [128, OF_COLS, 2], F32, kind="ExternalInput")
    ofidx = nc.dram_tensor("ofidx", [128, OF_TOK // 16], I16, kind="ExternalInput")
    grids = [
        nc.dram_tensor(f"grid{k}", [NSLOT, 128], F32, kind="ExternalOutput")
        for k in range(2)
    ]

    with tile.TileContext(nc) as tc:
        with tc.tile_pool(name="p", bufs=4) as pool:
            wcount = [0]

            def do_chunk(xsrc, mpsrc, idxsrc, col0, w):
                x_t = pool.tile([128, w, 64], F32, tag="x")
                mp_t = pool.tile([128, w, 2], F32, tag="mp")
                idx_t = pool.tile([128, w * 8], I16, tag="ix")
                pk_t = pool.tile([128, w, 128], F32, tag="pk")
                nc.sync.dma_start(out=x_t[:], in_=xsrc.ap()[:, col0:col0 + w, :])
                nc.sync.dma_start(out=mp_t[:], in_=mpsrc.ap()[:, col0:col0 + w, :])
                nc.sync.dma_start(
                    out=idx_t[:], in_=idxsrc.ap()[:, col0 * 8:(col0 + w) * 8]
                )
                for u in (0, 1):
                    nc.vector.tensor_tensor(
                        out=pk_t[:, :, 64 * u:64 * u + 64],
                        in0=x_t[:],
                        in1=mp_t[:, :, u:u + 1].broadcast_to([128, w, 64]),
                        op=mybir.AluOpType.mult,
                    )
                for j in range(w // 8):
                    g = grids[wcount[0] & 1]
                    wcount[0] += 1
                    nc.gpsimd.dma_scatter_add(
                        g.ap()[:],
                        pk_t[:, 8 * j:8 * j + 8, :],
                        idx_t[:, 64 * j:64 * j + 64],
                        WIN,
                        WIN,
                        128,
                    )

            def do_chunk_small(col0, w):
                """w cols, each col = one 128-token window."""
                x_t = pool.tile([128, w, 64], F32, tag="x")
                mp_t = pool.tile([128, w, 2], F32, tag="mp")
                idx_t = pool.tile([128, w * 8], I16, tag="ix")
                pk_t = pool.tile([128, w, 128], F32, tag="pk")
                nc.sync.dma_start(out=x_t[:], in_=ofx.ap()[:, col0:col0 + w, :])
                nc.sync.dma_start(out=mp_t[:], in_=ofmp.ap()[:, col0:col0 + w, :])
                nc.sync.dma_start(
                    out=idx_t[:], in_=ofidx.ap()[:, col0 * 8:(col0 + w) * 8]
                )
                for u in (0, 1):
                    nc.vector.tensor_tensor(
                        out=pk_t[:, :, 64 * u:64 * u + 64],
                        in0=x_t[:],
                        in1=mp_t[:, :, u:u + 1].broadcast_to([128, w, 64]),
                        op=mybir.AluOpType.mult,
                    )
                for j in range(w):
                    g = grids[wcount[0] & 1]
                    wcount[0] += 1
                    nc.gpsimd.dma_scatter_add(
                        g.ap()[:],
                        pk_t[:, j:j + 1, :],
                        idx_t[:, 8 * j:8 * j + 8],
                        128,
                        128,
                        128,
                    )

            # 30 chunks x 64 cols (8 windows) + 1 chunk x 32 cols (4 windows)
            for ci in range(30):
                do_chunk(xw, mpw, idxw, ci * 64, 64)
            do_chunk(xw, mpw, idxw, 30 * 64, 32)
            # overflow big windows: 48 windows = 6 chunks x 64 cols
            for ci in range(6):
                do_chunk(ofx, ofmp, ofidx, ci * 64, 64)
            # overflow small windows: 256 cols, 4 chunks x 64
            for ci in range(4):
                do_chunk_small(OF_BIG * 8 + ci * 64, 64)

    nc.compile()
    return nc


def _wrap16(tok):
    """token stream [T] -> [128, T//16] int16 (replicated across 8 groups)."""
    t16 = tok.reshape(-1, 16).T.astype(np.int16)      # [16, T//16]
    return np.tile(t16, (8, 1))


def _free_slot(used, w):
    s = (w * 977 + 13) % NSLOT
    while s in used:
        s = (s + 1) % NSLOT
    return s


def _prep_core(xb, slot, kept, par):
    """Build per-core device inputs. xb [NP,64] f32; slot/kept/par [NP]."""
    slot_p = np.zeros(TOKPAD, np.int64)
    kept_p = np.zeros(TOKPAD, bool)
    par_p = np.zeros(TOKPAD, np.int64)
    slot_p[:NP] = slot
    kept_p[:NP] = kept
    par_p[:NP] = par

    idx_main = np.zeros(TOKPAD, np.int64)
    m_main = np.zeros((TOKPAD, 2), np.float32)
    of_list = []   # (point_id, slot, parity)
    for w in range(NWIN):
        lo, hi = w * WIN, (w + 1) * WIN
        sl = slot_p[lo:hi]
        kp = kept_p[lo:hi]
        live = np.nonzero(kp)[0]
        _, first_pos = np.unique(sl[live], return_index=True)
        keepers = live[first_pos]
        dups = np.setdiff1d(live, keepers, assume_unique=False)
        used = set(sl[keepers].tolist())
        dead = _free_slot(used, w)
        idx_main[lo:hi] = dead
        idx_main[lo + keepers] = sl[keepers]
        m_main[lo + keepers, par_p[lo + keepers]] = 1.0
        for d in dups:
            of_list.append((lo + d, sl[d], par_p[lo + d]))

    # overflow: greedy re-window; window w holds at most one occurrence of a
    # slot. Mixed capacities: OF_BIG full windows then OF_SMALL 128-wide.
    n_w = OF_BIG + OF_SMALL
    caps = [WIN] * OF_BIG + [128] * OF_SMALL
    bases = np.concatenate([[0], np.cumsum(caps)])[:-1]
    of_windows = [[] for _ in range(n_w)]
    nxt = {}
    for rec in of_list:
        w = nxt.get(rec[1], 0)
        while w < n_w and len(of_windows[w]) >= caps[w]:
            w += 1
        assert w < n_w, f"overflow capacity exceeded ({len(of_list)} records)"
        of_windows[w].append(rec)
        nxt[rec[1]] = w + 1

    of_idx = np.zeros(OF_TOK, np.int64)
    of_m = np.zeros((OF_TOK, 2), np.float32)
    of_x = np.zeros((OF_TOK, 64), np.float32)
    for wi in range(n_w):
        recs = of_windows[wi]
        used = {r[1] for r in recs}
        dead = _free_slot(used, 10_000 + wi)
        b0 = bases[wi]
        of_idx[b0:b0 + caps[wi]] = dead
        for k, (pid, s, p_) in enumerate(recs):
            of_idx[b0 + k] = s
            of_m[b0 + k, p_] = 1.0
            of_x[b0 + k] = xb[pid]

    xpad = np.zeros((TOKPAD, 64), np.float32)
    xpad[:NP] = xb
    return {
        "xw": np.ascontiguousarray(
            xpad.reshape(COLS, 128, 64).transpose(1, 0, 2)),
        "mpw": np.ascontiguousarray(
            m_main.reshape(COLS, 128, 2).transpose(1, 0, 2)),
        "idxw": _wrap16(idx_main),
        "ofx": np.ascontiguousarray(
            of_x.reshape(OF_COLS, 128, 64).transpose(1, 0, 2)),
        "ofmp": np.ascontiguousarray(
            of_m.reshape(OF_COLS, 128, 2).transpose(1, 0, 2)),
        "ofidx": _wrap16(of_idx),
    }


def kernel(x, geom, dx, bx):
    global _nc_cache
    x = np.asarray(x, np.float32)
    geom = np.asarray(geom, np.float32)
    dx = np.asarray(dx, np.float32)
    bx = np.asarray(bx, np.float32)

    # exact f32 mirror of the reference voxelization
    off = (bx - dx / np.float32(2.0)).astype(np.float32)
    g = ((geom - off) / dx).astype(np.int32)       # trunc toward zero
    g = g.reshape(B, NP, 3)
    kept = ((g[..., 0] >= 0) & (g[..., 0] < NX)
            & (g[..., 1] >= 0) & (g[..., 1] < NY)
            & (g[..., 2] >= 0) & (g[..., 2] < 1))
    gx = g[..., 0].astype(np.int64)
    gy = g[..., 1].astype(np.int64)
    slot = np.where(kept, gx * 128 + gy // 2, 0)
    par = np.where(kept, gy & 1, 0)

    xf = x.reshape(B, NP, 64)
    in_maps = [
        _prep_core(xf[b], slot[b], kept[b], par[b]) for b in range(B)
    ]

    if _nc_cache is None:
        _nc_cache = _build()
    import time as _time
    _t0 = _time.perf_counter()
    res = run_bass_kernel_spmd(_nc_cache, in_maps, core_ids=list(range(8)))
    global LAST_DEVICE_CALL_S
    LAST_DEVICE_CALL_S = _time.perf_counter() - _t0

    out = np.empty((B, 64, NX, NY), np.float32)
    for b in range(B):
        gr = res.results[b]["grid0"] + res.results[b]["grid1"]   # [32768, 128]
        gr = gr.reshape(NX, NY // 2, 2, 64)         # gx, gy//2, gy&1, c
        out[b] = gr.transpose(3, 0, 1, 2).reshape(64, NX, NY)
    return out



# revision 5
# speedup vs baseline: 17.2786x; 17.2786x over previous
"""BEV voxel-pooling kernel for 8 TRN2 NeuronCores (data-parallel over batch).

Strategy (v2 — replaces the serialized dma_scatter_add chain):
  The axon-tunneled device call is transfer-bound (~39 MB/s tunnel, ~0.25 s
  fixed overhead), so the kernel minimizes bytes shipped:
    * host drops out-of-grid points (28.7%), sorts survivors by voxel id,
      and ships features as int8 with a per-point bf16 scale (measured
      end-to-end rel err 7.7e-3 vs 2e-2 budget);
    * the scatter itself is restructured so the device never scatters:
      voxels are split into 128 windows of 512 consecutive voxel ids. All
      points of a window are matmul-accumulated into one PSUM bank via
      equality one-hots: psum[64ch, 512segs] += x_tile[128pts, 64]^T @
      onehot[128pts, 512], then cast to bf16 and written densely to a
      feature-major grid gridT[64, 65536] — which reshapes to the final
      (64, 256, 256) output with no transpose.
  SPMD: all 8 cores run the same program; per-window tile counts are the
  max over cores, with per-core padding points (locs=-1) that match no
  one-hot column. Device output grid is bf16 (PSUM accumulate is f32).
"""
import numpy as np
import ml_dtypes

import concourse.bacc as bacc
import concourse.tile as tile
from concourse import mybir
from concourse.bass_utils import run_bass_kernel_spmd

F32 = mybir.dt.float32
BF16 = mybir.dt.bfloat16
I8 = mybir.dt.int8
I16 = mybir.dt.int16
I32 = mybir.dt.int32

B, N, D, H, W, C = 8, 6, 59, 16, 44, 64
NP = N * D * H * W            # 249216 points per batch/core
NX, NY = 256, 256
NSEG = NX * NY                # 65536 voxels per core
GW = 512                      # voxels per window (one PSUM bank free-dim)
NG = NSEG // GW               # 128 windows
CHUNK = 64                    # cols per load chunk

_nc_cache = {}


def _build(tg):
    """Build the SPMD program for per-window tile counts `tg` (len NG)."""
    cols = int(sum(tg))
    nc = bacc.Bacc("TRN2", target_bir_lowering=False, debug=False)
    q_h = nc.dram_tensor("q", [128, cols, C], I8, kind="ExternalInput")
    s_h = nc.dram_tensor("s", [128, cols, 1], BF16, kind="ExternalInput")
    l_h = nc.dram_tensor("l", [128, cols], I16, kind="ExternalInput")
    grid = nc.dram_tensor("gridT", [C, NSEG], BF16, kind="ExternalOutput")

    # col -> window id
    col2g = np.repeat(np.arange(NG), tg)
    assert len(col2g) == cols

    with tile.TileContext(nc) as tc:
        with tc.tile_pool(name="const", bufs=1) as cpool, \
             tc.tile_pool(name="io", bufs=3) as iopool, \
             tc.tile_pool(name="work", bufs=4) as wpool, \
             tc.tile_pool(name="stage", bufs=3) as spool, \
             tc.tile_pool(name="psum", bufs=2, space="PSUM") as ppool:
            iota_i = cpool.tile([128, GW], I32, tag="iota_i")
            iota_f = cpool.tile([128, GW], F32, tag="iota_f")
            nc.gpsimd.iota(iota_i[:], pattern=[[1, GW]], base=0,
                           channel_multiplier=0)
            nc.vector.tensor_copy(out=iota_f[:], in_=iota_i[:])

            xb_t = None
            lf_t = None
            psum_t = None
            for j in range(cols):
                if j % CHUNK == 0:
                    w = min(CHUNK, cols - j)
                    q_t = iopool.tile([128, w, C], I8, tag="q")
                    s_t = iopool.tile([128, w, 1], BF16, tag="s")
                    l_t = iopool.tile([128, w], I16, tag="l")
                    nc.sync.dma_start(out=q_t[:], in_=q_h.ap()[:, j:j + w, :])
                    nc.sync.dma_start(out=s_t[:], in_=s_h.ap()[:, j:j + w, :])
                    nc.sync.dma_start(out=l_t[:], in_=l_h.ap()[:, j:j + w])
                    xb_t = wpool.tile([128, w, C], BF16, tag="xb")
                    lf_t = wpool.tile([128, w], F32, tag="lf")
                    nc.vector.tensor_copy(out=xb_t[:], in_=q_t[:])
                    nc.vector.tensor_tensor(
                        out=xb_t[:], in0=xb_t[:],
                        in1=s_t[:, :, 0:1].broadcast_to([128, w, C]),
                        op=mybir.AluOpType.mult)
                    nc.vector.tensor_copy(out=lf_t[:], in_=l_t[:])
                jj = j % CHUNK
                g = col2g[j]
                first = (j == 0) or (col2g[j - 1] != g)
                last = (j == cols - 1) or (col2g[j + 1] != g)
                if first:
                    psum_t = ppool.tile([C, GW], F32, tag="acc")
                onehot = wpool.tile([128, GW], BF16, tag="oh")
                nc.vector.tensor_tensor(
                    out=onehot[:],
                    in0=lf_t[:, jj:jj + 1].broadcast_to([128, GW]),
                    in1=iota_f[:],
                    op=mybir.AluOpType.is_equal)
                nc.tensor.matmul(
                    out=psum_t[:], lhsT=xb_t[:, jj, :], rhs=onehot[:],
                    start=first, stop=last)
                if last:
                    stage = spool.tile([C, GW], BF16, tag="st")
                    nc.vector.tensor_copy(out=stage[:], in_=psum_t[:])
                    nc.sync.dma_start(
                        out=grid.ap()[:, g * GW:(g + 1) * GW], in_=stage[:])

    nc.compile()
    return nc


def _prep(xf, seg, kept):
    """Per-core host prep: sort kept points by voxel, window-bucket them.

    Returns (counts per window, sorted xs, sorted seg) for tg sizing pass."""
    k = np.nonzero(kept)[0]
    s = seg[k]
    order = np.argsort(s, kind="stable")
    ks = k[order]
    s = s[order]
    xs = xf[ks]
    cnt = np.bincount(s >> 9, minlength=NG)  # points per 512-voxel window
    return cnt, xs, s


def kernel(x, geom, dx, bx):
    x = np.asarray(x, np.float32)
    geom = np.asarray(geom, np.float32)
    dx = np.asarray(dx, np.float32)
    bx = np.asarray(bx, np.float32)

    # exact f32 mirror of the reference voxelization
    off = (bx - dx / np.float32(2.0)).astype(np.float32)
    g = ((geom - off) / dx).astype(np.int32).reshape(B, NP, 3)
    kept = ((g[..., 0] >= 0) & (g[..., 0] < NX)
            & (g[..., 1] >= 0) & (g[..., 1] < NY)
            & (g[..., 2] >= 0) & (g[..., 2] < 1))
    seg = g[..., 0].astype(np.int32) * NY + g[..., 1].astype(np.int32)

    xf = x.reshape(B, NP, C)
    prepped = [_prep(xf[b], seg[b], kept[b]) for b in range(B)]
    counts = np.stack([p[0] for p in prepped])          # [B, NG]
    tg = np.maximum(1, -(-counts.max(axis=0) // 128))   # tiles per window
    cols = int(tg.sum())
    colbase = np.concatenate([[0], np.cumsum(tg)])[:-1]

    in_maps = []
    for b in range(B):
        cnt, xs, s = prepped[b]
        npts = len(s)
        q = np.zeros((cols * 128, C), np.int8)
        sc = np.zeros(cols * 128, np.float32)
        lc = np.full(cols * 128, -1, np.int16)
        # destination token index for each sorted point: window w's points go
        # to tokens [colbase[w]*128, colbase[w]*128 + cnt[w])
        wstart = np.concatenate([[0], np.cumsum(cnt)])[:-1]
        tok = (colbase[s >> 9] * 128) + (np.arange(npts) - wstart[s >> 9])
        amax = np.abs(xs).max(axis=1)
        scale = np.maximum(amax / np.float32(127.0), 1e-8).astype(np.float32)
        q[tok] = np.clip(np.round(xs / scale[:, None]), -127, 127).astype(np.int8)
        sc[tok] = scale
        lc[tok] = (s & (GW - 1)).astype(np.int16)
        # token t -> (partition t%128, col t//128)
        in_maps.append({
            "q": np.ascontiguousarray(
                q.reshape(cols, 128, C).transpose(1, 0, 2)),
            "s": np.ascontiguousarray(
                sc.reshape(cols, 128).T.astype(ml_dtypes.bfloat16)
            ).reshape(128, cols, 1),
            "l": np.ascontiguousarray(lc.reshape(cols, 128).T),
        })

    key = tuple(tg.tolist())
    if key not in _nc_cache:
        _nc_cache.clear()
        _nc_cache[key] = _build(tg)
    import time as _time
    _t0 = _time.perf_counter()
    res = run_bass_kernel_spmd(_nc_cache[key], in_maps, core_ids=list(range(8)))
    global LAST_DEVICE_CALL_S
    LAST_DEVICE_CALL_S = _time.perf_counter() - _t0

    out = np.empty((B, C, NX, NY), np.float32)
    for b in range(B):
        out[b] = res.results[b]["gridT"].astype(np.float32).reshape(C, NX, NY)
    return out


# revision 11
# speedup vs baseline: 18.7451x; 1.0849x over previous
"""BEV voxel-pooling kernel for 8 TRN2 NeuronCores (data-parallel over batch).

Strategy (v2 — replaces the serialized dma_scatter_add chain):
  The axon-tunneled device call is transfer-bound (~39 MB/s tunnel, ~0.25 s
  fixed overhead), so the kernel minimizes bytes shipped:
    * host drops out-of-grid points (28.7%), sorts survivors by voxel id,
      and ships features as int8 with a per-point bf16 scale (measured
      end-to-end rel err 7.7e-3 vs 2e-2 budget);
    * the scatter itself is restructured so the device never scatters:
      voxels are split into 128 windows of 512 consecutive voxel ids. All
      points of a window are matmul-accumulated into one PSUM bank via
      equality one-hots: psum[64ch, 512segs] += x_tile[128pts, 64]^T @
      onehot[128pts, 512], then cast to bf16 and written densely to a
      feature-major grid gridT[64, 65536] — which reshapes to the final
      (64, 256, 256) output with no transpose.
  SPMD: all 8 cores run the same program; per-window tile counts are the
  max over cores, with per-core padding points (locs=-1) that match no
  one-hot column. Device output grid is bf16 (PSUM accumulate is f32).
"""
import numpy as np
import ml_dtypes

import concourse.bacc as bacc
import concourse.tile as tile
from concourse import mybir
from concourse.bass_utils import run_bass_kernel_spmd
from concourse.masks import make_identity

F32 = mybir.dt.float32
BF16 = mybir.dt.bfloat16
I8 = mybir.dt.int8
I16 = mybir.dt.int16
I32 = mybir.dt.int32

B, N, D, H, W, C = 8, 6, 59, 16, 44, 64
NP = N * D * H * W            # 249216 points per batch/core
NX, NY = 256, 256
NSEG = NX * NY                # 65536 voxels per core
GW = 512                      # voxels per window (one PSUM bank free-dim)
NG = NSEG // GW               # 128 windows
CHUNK = 64                    # cols per load chunk

_nc_cache = {}


def _build(tg):
    """Build the SPMD program for per-window tile counts `tg` (len NG)."""
    cols = int(sum(tg))
    nc = bacc.Bacc("TRN2", target_bir_lowering=False, debug=False)
    q_h = nc.dram_tensor("q", [128, cols, C], I8, kind="ExternalInput")
    s_h = nc.dram_tensor("s", [128, cols, 1], BF16, kind="ExternalInput")
    l_h = nc.dram_tensor("l", [128, cols], I16, kind="ExternalInput")
    # int8 grid + per-voxel bf16 scale; voxel (512*(col//4) + 128*(col%4) + p)
    # lives at gridq[p, col*64:col*64+64] with scale grids[p, col]
    gridq = nc.dram_tensor("gridq", [128, (NSEG // 128) * C], I8,
                           kind="ExternalOutput")
    grids = nc.dram_tensor("grids", [128, NSEG // 128], BF16,
                           kind="ExternalOutput")

    # col -> window id
    col2g = np.repeat(np.arange(NG), tg)
    assert len(col2g) == cols

    with tile.TileContext(nc) as tc:
        with tc.tile_pool(name="const", bufs=1) as cpool, \
             tc.tile_pool(name="io", bufs=3) as iopool, \
             tc.tile_pool(name="work", bufs=4) as wpool, \
             tc.tile_pool(name="stage", bufs=3) as spool, \
             tc.tile_pool(name="psum", bufs=2, space="PSUM") as ppool, \
             tc.tile_pool(name="psumt", bufs=4, space="PSUM") as tpool:
            iota_i = cpool.tile([128, GW], I32, tag="iota_i")
            iota_f = cpool.tile([128, GW], F32, tag="iota_f")
            nc.gpsimd.iota(iota_i[:], pattern=[[1, GW]], base=0,
                           channel_multiplier=0)
            nc.vector.tensor_copy(out=iota_f[:], in_=iota_i[:])
            ident = cpool.tile([C, C], F32, tag="ident")
            make_identity(nc, ident[:])
            gq_sb = cpool.tile([128, (NSEG // 128) * C], I8, tag="gq")
            gs_sb = cpool.tile([128, NSEG // 128], BF16, tag="gs")

            xb_t = None
            lf_t = None
            psum_t = None
            for j in range(cols):
                if j % CHUNK == 0:
                    w = min(CHUNK, cols - j)
                    q_t = iopool.tile([128, w, C], I8, tag="q")
                    s_t = iopool.tile([128, w, 1], BF16, tag="s")
                    l_t = iopool.tile([128, w], I16, tag="l")
                    nc.sync.dma_start(out=q_t[:], in_=q_h.ap()[:, j:j + w, :])
                    nc.sync.dma_start(out=s_t[:], in_=s_h.ap()[:, j:j + w, :])
                    nc.sync.dma_start(out=l_t[:], in_=l_h.ap()[:, j:j + w])
                    xb_t = wpool.tile([128, w, C], BF16, tag="xb")
                    lf_t = wpool.tile([128, w], F32, tag="lf")
                    nc.vector.tensor_copy(out=xb_t[:], in_=q_t[:])
                    nc.vector.tensor_tensor(
                        out=xb_t[:], in0=xb_t[:],
                        in1=s_t[:, :, 0:1].broadcast_to([128, w, C]),
                        op=mybir.AluOpType.mult)
                    nc.vector.tensor_copy(out=lf_t[:], in_=l_t[:])
                jj = j % CHUNK
                g = col2g[j]
                first = (j == 0) or (col2g[j - 1] != g)
                last = (j == cols - 1) or (col2g[j + 1] != g)
                if first:
                    psum_t = ppool.tile([C, GW], F32, tag="acc")
                onehot = wpool.tile([128, GW], BF16, tag="oh")
                nc.vector.tensor_tensor(
                    out=onehot[:],
                    in0=lf_t[:, jj:jj + 1].broadcast_to([128, GW]),
                    in1=iota_f[:],
                    op=mybir.AluOpType.is_equal)
                nc.tensor.matmul(
                    out=psum_t[:], lhsT=xb_t[:, jj, :], rhs=onehot[:],
                    start=first, stop=last)
                if last:
                    # quantize this window's 512 voxels to int8 + bf16 scale:
                    # transpose to voxel-major 128-row chunks, per-voxel absmax
                    sbf = spool.tile([C, GW], F32, tag="sf")
                    nc.vector.tensor_copy(out=sbf[:], in_=psum_t[:])
                    for t in range(GW // 128):
                        col = g * (GW // 128) + t
                        trp = tpool.tile([128, C], F32, tag="tr")
                        nc.tensor.transpose(
                            out=trp[:], in_=sbf[:, 128 * t:128 * (t + 1)],
                            identity=ident[:])
                        rmax = wpool.tile([128, 1], F32, tag="rm")
                        nc.vector.tensor_reduce(
                            out=rmax[:], in_=trp[:], axis=mybir.AxisListType.X,
                            op=mybir.AluOpType.max, apply_absolute_value=True)
                        nc.vector.tensor_scalar_max(
                            out=rmax[:], in0=rmax[:], scalar1=1e-8)
                        inv = wpool.tile([128, 1], F32, tag="inv")
                        nc.vector.reciprocal(out=inv[:], in_=rmax[:])
                        nc.vector.tensor_scalar(
                            out=gq_sb[:, col * C:(col + 1) * C], in0=trp[:],
                            scalar1=inv[:, 0:1], scalar2=127.0,
                            op0=mybir.AluOpType.mult, op1=mybir.AluOpType.mult)
                        nc.vector.tensor_scalar(
                            out=gs_sb[:, col:col + 1], in0=rmax[:],
                            scalar1=float(np.float32(1.0 / 127.0)),
                            scalar2=None, op0=mybir.AluOpType.mult)
            nc.sync.dma_start(out=gridq.ap()[:], in_=gq_sb[:])
            nc.sync.dma_start(out=grids.ap()[:], in_=gs_sb[:])

    nc.compile()
    return nc


def _prep(xf, seg, kept):
    """Per-core host prep: sort kept points by voxel, window-bucket them.

    Returns (counts per window, sorted xs, sorted seg) for tg sizing pass."""
    k = np.nonzero(kept)[0]
    s = seg[k]
    order = np.argsort(s, kind="stable")
    ks = k[order]
    s = s[order]
    xs = xf[ks]
    cnt = np.bincount(s >> 9, minlength=NG)  # points per 512-voxel window
    return cnt, xs, s


def kernel(x, geom, dx, bx):
    x = np.asarray(x, np.float32)
    geom = np.asarray(geom, np.float32)
    dx = np.asarray(dx, np.float32)
    bx = np.asarray(bx, np.float32)

    # exact f32 mirror of the reference voxelization
    off = (bx - dx / np.float32(2.0)).astype(np.float32)
    g = ((geom - off) / dx).astype(np.int32).reshape(B, NP, 3)
    kept = ((g[..., 0] >= 0) & (g[..., 0] < NX)
            & (g[..., 1] >= 0) & (g[..., 1] < NY)
            & (g[..., 2] >= 0) & (g[..., 2] < 1))
    seg = g[..., 0].astype(np.int32) * NY + g[..., 1].astype(np.int32)

    xf = x.reshape(B, NP, C)
    prepped = [_prep(xf[b], seg[b], kept[b]) for b in range(B)]
    counts = np.stack([p[0] for p in prepped])          # [B, NG]
    tg = np.maximum(1, -(-counts.max(axis=0) // 128))   # tiles per window
    cols = int(tg.sum())
    colbase = np.concatenate([[0], np.cumsum(tg)])[:-1]

    in_maps = []
    for b in range(B):
        cnt, xs, s = prepped[b]
        npts = len(s)
        q = np.zeros((cols * 128, C), np.int8)
        sc = np.zeros(cols * 128, np.float32)
        lc = np.full(cols * 128, -1, np.int16)
        # destination token index for each sorted point: window w's points go
        # to tokens [colbase[w]*128, colbase[w]*128 + cnt[w])
        wstart = np.concatenate([[0], np.cumsum(cnt)])[:-1]
        tok = (colbase[s >> 9] * 128) + (np.arange(npts) - wstart[s >> 9])
        amax = np.abs(xs).max(axis=1)
        scale = np.maximum(amax / np.float32(127.0), 1e-8).astype(np.float32)
        q[tok] = np.clip(np.round(xs / scale[:, None]), -127, 127).astype(np.int8)
        sc[tok] = scale
        lc[tok] = (s & (GW - 1)).astype(np.int16)
        # token t -> (partition t%128, col t//128)
        in_maps.append({
            "q": np.ascontiguousarray(
                q.reshape(cols, 128, C).transpose(1, 0, 2)),
            "s": np.ascontiguousarray(
                sc.reshape(cols, 128).T.astype(ml_dtypes.bfloat16)
            ).reshape(128, cols, 1),
            "l": np.ascontiguousarray(lc.reshape(cols, 128).T),
        })

    key = tuple(tg.tolist())
    if key not in _nc_cache:
        _nc_cache.clear()
        _nc_cache[key] = _build(tg)
    import time as _time
    _t0 = _time.perf_counter()
    res = run_bass_kernel_spmd(_nc_cache[key], in_maps, core_ids=list(range(8)))
    global LAST_DEVICE_CALL_S
    LAST_DEVICE_CALL_S = _time.perf_counter() - _t0

    out = np.empty((B, C, NX, NY), np.float32)
    ncol = NSEG // 128
    for b in range(B):
        q8 = res.results[b]["gridq"].astype(np.float32).reshape(128, ncol, C)
        sc = res.results[b]["grids"].astype(np.float32)
        val = q8 * sc[:, :, None]                       # [p, col, c]
        # voxel = 512*(col//4) + 128*(col%4) + p
        val = val.reshape(128, NG, GW // 128, C).transpose(1, 2, 0, 3)
        out[b] = val.reshape(NSEG, C).T.reshape(C, NX, NY)
    return out


# revision 14
# speedup vs baseline: 23.3621x; 1.2463x over previous
"""BEV voxel-pooling kernel for 8 TRN2 NeuronCores (data-parallel over batch).

Strategy (v4):
  The axon-tunneled device call is transfer-bound (~14 us/MB serialization +
  ~11 us/MB wire for incompressible bytes, ~0.25 s fixed), so the kernel
  minimizes bytes shipped:
    * host drops out-of-grid points (28.7%), sorts survivors by voxel id,
      and ships features as int8 with a per-point bf16 scale (measured
      end-to-end rel err ~9e-3 vs 2e-2 budget);
    * the scatter is restructured so the device never scatters: voxels are
      split into 128 windows of 512 consecutive voxel ids. All points of a
      window are matmul-accumulated into one PSUM bank via equality
      one-hots: psum[64ch, 512vox] += x_tile[128pts, 64]^T @
      onehot[128pts, 512]. Token packing is not tile-aligned (window w owns
      tokens [O_w, O_w + maxcnt_w)); a 128-point tile overlapping k windows
      issues k matmuls with window-offset iota constants.
    * each window is then transposed (PE) to voxel-major 128-row chunks and
      quantized to int8 with a per-voxel bf16 scale before a single dense
      DMA out — output is 4.3 MB/core instead of 16.8 MB f32.
  SPMD: all 8 cores run one program; per-window token capacity is the max
  over cores, per-core padding points have q=0/s=0 so they add nothing.
  A persistent XLA compilation cache avoids the ~1.4 s/call re-jit of
  run_bass_via_pjrt's fresh closure.
"""
import numpy as np
import ml_dtypes

import jax

jax.config.update("jax_compilation_cache_dir", "/tmp/.bev_jax_cache")
jax.config.update("jax_persistent_cache_min_compile_time_secs", 0.0)
jax.config.update("jax_persistent_cache_min_entry_size_bytes", 0)

import concourse.bacc as bacc
import concourse.tile as tile
from concourse import mybir
from concourse.bass_utils import run_bass_kernel_spmd
from concourse.masks import make_identity

F32 = mybir.dt.float32
BF16 = mybir.dt.bfloat16
I8 = mybir.dt.int8
I16 = mybir.dt.int16
I32 = mybir.dt.int32

B, N, D, H, W, C = 8, 6, 59, 16, 44, 64
NP = N * D * H * W            # 249216 points per batch/core
NX, NY = 256, 256
NSEG = NX * NY                # 65536 voxels per core
GW = 512                      # voxels per window (one PSUM bank free-dim)
NG = NSEG // GW               # 128 windows
CHUNK = 64                    # cols per load chunk
NCOL_OUT = NSEG // 128        # 512 output chunks of 128 voxels

_nc_cache = {}


def _schedule(maxcnt):
    """Static schedule from per-window token capacities."""
    O = np.concatenate([[0], np.cumsum(maxcnt)]).astype(np.int64)
    tok = int(O[-1])
    cols = -(-tok // 128)
    # first window owning token 128*j (skipping empty windows)
    gfirst = np.searchsorted(O[1:], np.arange(cols) * 128, side="right")
    # per tile: list of (window, k=window-gfirst, start_tile, stop_tile)
    tiles = []
    for j in range(cols):
        lo, hi = j * 128, min(j * 128 + 128, tok)
        ov = []
        g = int(gfirst[j])
        while g < NG and O[g] < hi:
            if maxcnt[g] > 0:
                ov.append((g, g - int(gfirst[j]),
                           O[g] >= lo, O[g + 1] <= hi or j == cols - 1))
            g += 1
        tiles.append(ov)
    maxk = max((k for ov in tiles for (_, k, _, _) in ov), default=0) + 1
    return O, tok, cols, tiles, maxk


def _build(maxcnt):
    O, tok, cols, tiles, maxk = _schedule(maxcnt)
    nc = bacc.Bacc("TRN2", target_bir_lowering=False, debug=False)
    q_h = nc.dram_tensor("q", [128, cols, C], I8, kind="ExternalInput")
    s_h = nc.dram_tensor("s", [128, cols, 1], BF16, kind="ExternalInput")
    l_h = nc.dram_tensor("l", [128, cols], I16, kind="ExternalInput")
    # int8 grid + per-voxel bf16 scale; voxel 128*col + p lives at
    # gridq[p, col*64:(col+1)*64] with scale grids[p, col]
    gridq = nc.dram_tensor("gridq", [128, NCOL_OUT * C], I8,
                           kind="ExternalOutput")
    grids = nc.dram_tensor("grids", [128, NCOL_OUT], BF16,
                           kind="ExternalOutput")

    with tile.TileContext(nc) as tc:
        with tc.tile_pool(name="const", bufs=1) as cpool, \
             tc.tile_pool(name="io", bufs=3) as iopool, \
             tc.tile_pool(name="work", bufs=4) as wpool, \
             tc.tile_pool(name="stage", bufs=3) as spool, \
             tc.tile_pool(name="psum", bufs=3, space="PSUM") as ppool, \
             tc.tile_pool(name="psumt", bufs=2, space="PSUM") as tpool:
            iotas = []
            for k in range(maxk):
                ii = cpool.tile([128, GW], I32, tag=f"ii{k}")
                if_ = cpool.tile([128, GW], F32, tag=f"if{k}")
                nc.gpsimd.iota(ii[:], pattern=[[1, GW]], base=k * GW,
                               channel_multiplier=0)
                nc.vector.tensor_copy(out=if_[:], in_=ii[:])
                iotas.append(if_)
            ident = cpool.tile([C, C], F32, tag="ident")
            make_identity(nc, ident[:])
            gq_sb = cpool.tile([128, NCOL_OUT * C], I8, tag="gq")
            gs_sb = cpool.tile([128, NCOL_OUT], BF16, tag="gs")
            nc.gpsimd.memset(gq_sb[:], 0)
            nc.gpsimd.memset(gs_sb[:], 0)

            def quantize_window(g, psum_t):
                # transpose to voxel-major chunks, per-voxel int8 + bf16 scale
                sbf = spool.tile([C, GW], F32, tag="sf")
                nc.vector.tensor_copy(out=sbf[:], in_=psum_t[:])
                for t in range(GW // 128):
                    col = g * (GW // 128) + t
                    trp = tpool.tile([128, C], F32, tag="tr")
                    nc.tensor.transpose(
                        out=trp[:], in_=sbf[:, 128 * t:128 * (t + 1)],
                        identity=ident[:])
                    rmax = wpool.tile([128, 1], F32, tag="rm")
                    nc.vector.tensor_reduce(
                        out=rmax[:], in_=trp[:], axis=mybir.AxisListType.X,
                        op=mybir.AluOpType.max, apply_absolute_value=True)
                    nc.vector.tensor_scalar_max(
                        out=rmax[:], in0=rmax[:], scalar1=1e-8)
                    inv = wpool.tile([128, 1], F32, tag="inv")
                    nc.vector.reciprocal(out=inv[:], in_=rmax[:])
                    nc.vector.tensor_scalar(
                        out=gq_sb[:, col * C:(col + 1) * C], in0=trp[:],
                        scalar1=inv[:, 0:1], scalar2=127.0,
                        op0=mybir.AluOpType.mult, op1=mybir.AluOpType.mult)
                    nc.vector.tensor_scalar(
                        out=gs_sb[:, col:col + 1], in0=rmax[:],
                        scalar1=float(np.float32(1.0 / 127.0)),
                        scalar2=None, op0=mybir.AluOpType.mult)

            psums = {}
            xb_t = None
            lf_t = None
            for j in range(cols):
                if j % CHUNK == 0:
                    w = min(CHUNK, cols - j)
                    q_t = iopool.tile([128, w, C], I8, tag="q")
                    s_t = iopool.tile([128, w, 1], BF16, tag="s")
                    l_t = iopool.tile([128, w], I16, tag="l")
                    nc.sync.dma_start(out=q_t[:], in_=q_h.ap()[:, j:j + w, :])
                    nc.sync.dma_start(out=s_t[:], in_=s_h.ap()[:, j:j + w, :])
                    nc.sync.dma_start(out=l_t[:], in_=l_h.ap()[:, j:j + w])
                    xb_t = wpool.tile([128, w, C], BF16, tag="xb")
                    lf_t = wpool.tile([128, w], F32, tag="lf")
                    nc.vector.tensor_copy(out=xb_t[:], in_=q_t[:])
                    nc.vector.tensor_tensor(
                        out=xb_t[:], in0=xb_t[:],
                        in1=s_t[:, :, 0:1].broadcast_to([128, w, C]),
                        op=mybir.AluOpType.mult)
                    nc.vector.tensor_copy(out=lf_t[:], in_=l_t[:])
                jj = j % CHUNK
                for (g, k, first, last) in tiles[j]:
                    if first:
                        psums[g] = ppool.tile([C, GW], F32, tag="acc",
                                              name=f"acc{g}")
                    onehot = wpool.tile([128, GW], BF16, tag="oh")
                    nc.vector.tensor_tensor(
                        out=onehot[:],
                        in0=lf_t[:, jj:jj + 1].broadcast_to([128, GW]),
                        in1=iotas[k][:],
                        op=mybir.AluOpType.is_equal)
                    nc.tensor.matmul(
                        out=psums[g][:], lhsT=xb_t[:, jj, :], rhs=onehot[:],
                        start=first, stop=last)
                    if last:
                        quantize_window(g, psums.pop(g))
            nc.sync.dma_start(out=gridq.ap()[:], in_=gq_sb[:])
            nc.sync.dma_start(out=grids.ap()[:], in_=gs_sb[:])

    nc.compile()
    return nc


def kernel(x, geom, dx, bx):
    x = np.asarray(x, np.float32)
    geom = np.asarray(geom, np.float32)
    dx = np.asarray(dx, np.float32)
    bx = np.asarray(bx, np.float32)

    # exact f32 mirror of the reference voxelization
    off = (bx - dx / np.float32(2.0)).astype(np.float32)
    g = ((geom - off) / dx).astype(np.int32).reshape(B, NP, 3)
    kept = ((g[..., 0] >= 0) & (g[..., 0] < NX)
            & (g[..., 1] >= 0) & (g[..., 1] < NY)
            & (g[..., 2] >= 0) & (g[..., 2] < 1))
    seg = g[..., 0].astype(np.int32) * NY + g[..., 1].astype(np.int32)

    xf = x.reshape(B, NP, C)
    per_core = []
    counts = np.zeros((B, NG), np.int64)
    for b in range(B):
        k = np.nonzero(kept[b])[0]
        s = seg[b][k]
        order = np.argsort(s, kind="stable")
        k = k[order]
        s = s[order]
        counts[b] = np.bincount(s >> 9, minlength=NG)
        per_core.append((k, s))
    maxcnt = counts.max(axis=0)
    O = np.concatenate([[0], np.cumsum(maxcnt)]).astype(np.int64)
    tok = int(O[-1])
    cols = -(-tok // 128)
    # window owning each tile's first token -> per-point iota base
    gfirst = np.searchsorted(O[1:], np.arange(cols) * 128, side="right")

    in_maps = []
    for b in range(B):
        k, s = per_core[b]
        npts = len(s)
        xs = xf[b][k]
        q = np.zeros((cols * 128, C), np.int8)
        sc = np.zeros(cols * 128, np.float32)
        lc = np.zeros(cols * 128, np.int16)
        wstart = np.concatenate([[0], np.cumsum(counts[b])])[:-1]
        gi = (s >> 9).astype(np.int64)
        tokidx = O[gi] + (np.arange(npts) - wstart[gi])
        amax = np.abs(xs).max(axis=1)
        scale = np.maximum(amax / np.float32(127.0), 1e-8).astype(np.float32)
        q[tokidx] = np.clip(np.round(xs / scale[:, None]), -127, 127
                            ).astype(np.int8)
        sc[tokidx] = scale
        # local voxel id relative to the first window of the point's tile
        lc[tokidx] = (s - (gfirst[tokidx >> 7] << 9)).astype(np.int16)
        in_maps.append({
            "q": np.ascontiguousarray(
                q.reshape(cols, 128, C).transpose(1, 0, 2)),
            "s": np.ascontiguousarray(
                sc.reshape(cols, 128).T.astype(ml_dtypes.bfloat16)
            ).reshape(128, cols, 1),
            "l": np.ascontiguousarray(lc.reshape(cols, 128).T),
        })

    key = tuple(maxcnt.tolist())
    if key not in _nc_cache:
        _nc_cache.clear()
        _nc_cache[key] = _build(maxcnt)
    import time as _time
    _t0 = _time.perf_counter()
    res = run_bass_kernel_spmd(_nc_cache[key], in_maps, core_ids=list(range(8)))
    global LAST_DEVICE_CALL_S
    LAST_DEVICE_CALL_S = _time.perf_counter() - _t0

    out = np.empty((B, C, NX, NY), np.float32)
    for b in range(B):
        q8 = res.results[b]["gridq"].astype(np.float32).reshape(128, NCOL_OUT, C)
        sc = res.results[b]["grids"].astype(np.float32)
        val = q8 * sc[:, :, None]                       # [p, col, c]
        # voxel = 128*col + p
        out[b] = val.transpose(1, 0, 2).reshape(NSEG, C).T.reshape(C, NX, NY)
    return out


# revision 15
# speedup vs baseline: 25.7108x; 1.1005x over previous
"""BEV voxel-pooling kernel for 8 TRN2 NeuronCores (data-parallel over batch).

Strategy (v4):
  The axon-tunneled device call is transfer-bound (~14 us/MB serialization +
  ~11 us/MB wire for incompressible bytes, ~0.25 s fixed), so the kernel
  minimizes bytes shipped:
    * host drops out-of-grid points (28.7%), sorts survivors by voxel id,
      and ships features as int8 with a per-point bf16 scale (measured
      end-to-end rel err ~9e-3 vs 2e-2 budget);
    * the scatter is restructured so the device never scatters: voxels are
      split into 128 windows of 512 consecutive voxel ids. All points of a
      window are matmul-accumulated into one PSUM bank via equality
      one-hots: psum[64ch, 512vox] += x_tile[128pts, 64]^T @
      onehot[128pts, 512]. Token packing is not tile-aligned (window w owns
      tokens [O_w, O_w + maxcnt_w)); a 128-point tile overlapping k windows
      issues k matmuls with window-offset iota constants.
    * each window is then transposed (PE) to voxel-major 128-row chunks and
      quantized to int8 with a per-voxel bf16 scale before a single dense
      DMA out — output is 4.3 MB/core instead of 16.8 MB f32.
  SPMD: all 8 cores run one program; per-window token capacity is the max
  over cores, per-core padding points have q=0/s=0 so they add nothing.
  A persistent XLA compilation cache avoids the ~1.4 s/call re-jit of
  run_bass_via_pjrt's fresh closure.
"""
import numpy as np
import ml_dtypes

import jax

jax.config.update("jax_compilation_cache_dir", "/tmp/.bev_jax_cache")
jax.config.update("jax_persistent_cache_min_compile_time_secs", 0.0)
jax.config.update("jax_persistent_cache_min_entry_size_bytes", 0)

import concourse.bacc as bacc
import concourse.tile as tile
from concourse import mybir
from concourse.bass_utils import run_bass_kernel_spmd
from concourse.masks import make_identity

F32 = mybir.dt.float32
BF16 = mybir.dt.bfloat16
I8 = mybir.dt.int8
I16 = mybir.dt.int16
I32 = mybir.dt.int32

B, N, D, H, W, C = 8, 6, 59, 16, 44, 64
NP = N * D * H * W            # 249216 points per batch/core
NX, NY = 256, 256
NSEG = NX * NY                # 65536 voxels per core
GW = 512                      # voxels per window (one PSUM bank free-dim)
NG = NSEG // GW               # 128 windows
CHUNK = 64                    # cols per load chunk
NCOL_OUT = NSEG // 128        # 512 output chunks of 128 voxels

_nc_cache = {}


def _schedule(maxcnt):
    """Static schedule from per-window token capacities."""
    O = np.concatenate([[0], np.cumsum(maxcnt)]).astype(np.int64)
    tok = int(O[-1])
    cols = -(-tok // 128)
    # first window owning token 128*j (skipping empty windows)
    gfirst = np.searchsorted(O[1:], np.arange(cols) * 128, side="right")
    # per tile: list of (window, k=window-gfirst, start_tile, stop_tile)
    tiles = []
    for j in range(cols):
        lo, hi = j * 128, min(j * 128 + 128, tok)
        ov = []
        g = int(gfirst[j])
        while g < NG and O[g] < hi:
            if maxcnt[g] > 0:
                ov.append((g, g - int(gfirst[j]),
                           O[g] >= lo, O[g + 1] <= hi or j == cols - 1))
            g += 1
        tiles.append(ov)
    maxk = max((k for ov in tiles for (_, k, _, _) in ov), default=0) + 1
    return O, tok, cols, tiles, maxk


def _build(maxcnt):
    O, tok, cols, tiles, maxk = _schedule(maxcnt)
    nc = bacc.Bacc("TRN2", target_bir_lowering=False, debug=False)
    q_h = nc.dram_tensor("q", [128, cols, C], I8, kind="ExternalInput")
    s_h = nc.dram_tensor("s", [128, cols, 1], BF16, kind="ExternalInput")
    l_h = nc.dram_tensor("l", [128, cols], I16, kind="ExternalInput")
    # int8 grid + per-voxel bf16 scale; voxel 128*col + p lives at
    # gridq[p, col*64:(col+1)*64] with scale grids[p, col]
    gridq = nc.dram_tensor("gridq", [128, NCOL_OUT * C], I8,
                           kind="ExternalOutput")
    grids = nc.dram_tensor("grids", [128, NCOL_OUT], BF16,
                           kind="ExternalOutput")

    with tile.TileContext(nc) as tc:
        with tc.tile_pool(name="const", bufs=1) as cpool, \
             tc.tile_pool(name="io", bufs=3) as iopool, \
             tc.tile_pool(name="work", bufs=4) as wpool, \
             tc.tile_pool(name="stage", bufs=3) as spool, \
             tc.tile_pool(name="psum", bufs=3, space="PSUM") as ppool, \
             tc.tile_pool(name="psumt", bufs=2, space="PSUM") as tpool:
            iotas = []
            for k in range(maxk):
                ii = cpool.tile([128, GW], I32, tag=f"ii{k}")
                if_ = cpool.tile([128, GW], F32, tag=f"if{k}")
                nc.gpsimd.iota(ii[:], pattern=[[1, GW]], base=k * GW,
                               channel_multiplier=0)
                nc.vector.tensor_copy(out=if_[:], in_=ii[:])
                iotas.append(if_)
            ident = cpool.tile([C, C], F32, tag="ident")
            make_identity(nc, ident[:])
            gq_sb = cpool.tile([128, NCOL_OUT * C], I8, tag="gq")
            gs_sb = cpool.tile([128, NCOL_OUT], BF16, tag="gs")
            nc.gpsimd.memset(gq_sb[:], 0)
            nc.gpsimd.memset(gs_sb[:], 0)

            def quantize_window(g, psum_t):
                # transpose to voxel-major chunks, per-voxel int8 + bf16 scale
                sbf = spool.tile([C, GW], F32, tag="sf")
                nc.vector.tensor_copy(out=sbf[:], in_=psum_t[:])
                for t in range(GW // 128):
                    col = g * (GW // 128) + t
                    trp = tpool.tile([128, C], F32, tag="tr")
                    nc.tensor.transpose(
                        out=trp[:], in_=sbf[:, 128 * t:128 * (t + 1)],
                        identity=ident[:])
                    rmax = wpool.tile([128, 1], F32, tag="rm")
                    nc.vector.tensor_reduce(
                        out=rmax[:], in_=trp[:], axis=mybir.AxisListType.X,
                        op=mybir.AluOpType.max, apply_absolute_value=True)
                    nc.vector.tensor_scalar_max(
                        out=rmax[:], in0=rmax[:], scalar1=1e-8)
                    inv = wpool.tile([128, 1], F32, tag="inv")
                    nc.vector.reciprocal(out=inv[:], in_=rmax[:])
                    nc.vector.tensor_scalar(
                        out=gq_sb[:, col * C:(col + 1) * C], in0=trp[:],
                        scalar1=inv[:, 0:1], scalar2=127.0,
                        op0=mybir.AluOpType.mult, op1=mybir.AluOpType.mult)
                    nc.vector.tensor_scalar(
                        out=gs_sb[:, col:col + 1], in0=rmax[:],
                        scalar1=float(np.float32(1.0 / 127.0)),
                        scalar2=None, op0=mybir.AluOpType.mult)

            psums = {}
            xb_t = None
            lf_t = None
            for j in range(cols):
                if j % CHUNK == 0:
                    w = min(CHUNK, cols - j)
                    q_t = iopool.tile([128, w, C], I8, tag="q")
                    s_t = iopool.tile([128, w, 1], BF16, tag="s")
                    l_t = iopool.tile([128, w], I16, tag="l")
                    nc.sync.dma_start(out=q_t[:], in_=q_h.ap()[:, j:j + w, :])
                    nc.sync.dma_start(out=s_t[:], in_=s_h.ap()[:, j:j + w, :])
                    nc.sync.dma_start(out=l_t[:], in_=l_h.ap()[:, j:j + w])
                    xb_t = wpool.tile([128, w, C], BF16, tag="xb")
                    lf_t = wpool.tile([128, w], F32, tag="lf")
                    nc.vector.tensor_copy(out=xb_t[:], in_=q_t[:])
                    nc.vector.tensor_tensor(
                        out=xb_t[:], in0=xb_t[:],
                        in1=s_t[:, :, 0:1].broadcast_to([128, w, C]),
                        op=mybir.AluOpType.mult)
                    nc.vector.tensor_copy(out=lf_t[:], in_=l_t[:])
                jj = j % CHUNK
                for (g, k, first, last) in tiles[j]:
                    if first:
                        psums[g] = ppool.tile([C, GW], F32, tag="acc",
                                              name=f"acc{g}")
                    onehot = wpool.tile([128, GW], BF16, tag="oh")
                    nc.vector.tensor_tensor(
                        out=onehot[:],
                        in0=lf_t[:, jj:jj + 1].broadcast_to([128, GW]),
                        in1=iotas[k][:],
                        op=mybir.AluOpType.is_equal)
                    nc.tensor.matmul(
                        out=psums[g][:], lhsT=xb_t[:, jj, :], rhs=onehot[:],
                        start=first, stop=last)
                    if last:
                        quantize_window(g, psums.pop(g))
            nc.sync.dma_start(out=gridq.ap()[:], in_=gq_sb[:])
            nc.sync.dma_start(out=grids.ap()[:], in_=gs_sb[:])

    nc.compile()
    return nc


def kernel(x, geom, dx, bx):
    x = np.asarray(x, np.float32)
    geom = np.asarray(geom, np.float32)
    dx = np.asarray(dx, np.float32)
    bx = np.asarray(bx, np.float32)

    # exact f32 mirror of the reference voxelization
    off = (bx - dx / np.float32(2.0)).astype(np.float32)
    g = ((geom - off) / dx).astype(np.int32).reshape(B, NP, 3)
    kept = ((g[..., 0] >= 0) & (g[..., 0] < NX)
            & (g[..., 1] >= 0) & (g[..., 1] < NY)
            & (g[..., 2] >= 0) & (g[..., 2] < 1))
    seg = g[..., 0].astype(np.int32) * NY + g[..., 1].astype(np.int32)

    xf = x.reshape(B, NP, C)
    per_core = []
    counts = np.zeros((B, NG), np.int64)
    for b in range(B):
        k = np.nonzero(kept[b])[0]
        s = seg[b][k]
        order = np.argsort(s, kind="stable")
        k = k[order]
        s = s[order]
        counts[b] = np.bincount(s >> 9, minlength=NG)
        per_core.append((k, s))
    maxcnt = counts.max(axis=0)
    O = np.concatenate([[0], np.cumsum(maxcnt)]).astype(np.int64)
    tok = int(O[-1])
    cols = -(-tok // 128)
    # window owning each tile's first token -> per-point iota base
    gfirst = np.searchsorted(O[1:], np.arange(cols) * 128, side="right")

    in_maps = []
    for b in range(B):
        k, s = per_core[b]
        npts = len(s)
        xs = xf[b][k]
        q = np.zeros((cols * 128, C), np.int8)
        sc = np.zeros(cols * 128, np.float32)
        lc = np.zeros(cols * 128, np.int16)
        wstart = np.concatenate([[0], np.cumsum(counts[b])])[:-1]
        gi = (s >> 9).astype(np.int64)
        tokidx = O[gi] + (np.arange(npts) - wstart[gi])
        amax = np.abs(xs).max(axis=1)
        scale = np.maximum(amax / np.float32(127.0), 1e-8).astype(np.float32)
        q[tokidx] = np.clip(np.round(xs / scale[:, None]), -127, 127
                            ).astype(np.int8)
        sc[tokidx] = scale
        # local voxel id relative to the first window of the point's tile
        lrel = s.astype(np.int64) - (gfirst[tokidx >> 7] << 9)
        assert 0 <= lrel.min() and lrel.max() < 32768, "l overflows int16"
        lc[tokidx] = lrel.astype(np.int16)
        in_maps.append({
            "q": np.ascontiguousarray(
                q.reshape(cols, 128, C).transpose(1, 0, 2)),
            "s": np.ascontiguousarray(
                sc.reshape(cols, 128).T.astype(ml_dtypes.bfloat16)
            ).reshape(128, cols, 1),
            "l": np.ascontiguousarray(lc.reshape(cols, 128).T),
        })

    key = tuple(maxcnt.tolist())
    if key not in _nc_cache:
        _nc_cache.clear()
        _nc_cache[key] = _build(maxcnt)
    import time as _time
    _t0 = _time.perf_counter()
    res = run_bass_kernel_spmd(_nc_cache[key], in_maps, core_ids=list(range(8)))
    global LAST_DEVICE_CALL_S
    LAST_DEVICE_CALL_S = _time.perf_counter() - _t0

    out = np.empty((B, C, NX, NY), np.float32)
    for b in range(B):
        q8 = res.results[b]["gridq"].astype(np.float32).reshape(128, NCOL_OUT, C)
        sc = res.results[b]["grids"].astype(np.float32)
        val = q8 * sc[:, :, None]                       # [p, col, c]
        # voxel = 128*col + p
        out[b] = val.transpose(1, 0, 2).reshape(NSEG, C).T.reshape(C, NX, NY)
    return out
